# revision 20
# baseline (speedup 1.0000x reference)
"""Trainium2 Bass kernel for nn_Listener (LSTM listener + dense encoders).

Reference computation (per full batch B=512):
    emb = embed_table[message]                       # [B, T, 512]
    LSTM over T=128 steps, HIDDEN=1024:
        gated = [x_t, h] @ W_cell + b_cell           # [B, 4096] (i, g, f, o)
        f = sigmoid(f + 1); c = f*c + sigmoid(i)*tanh(g); h = sigmoid(o)*tanh(c)
    images_encoded = images @ W_img + b_img          # [B, 1024]
    hidden_encoded = h @ W_hid + b_hid               # [B, 1024]
    returns (images_encoded, hidden_encoded)

Strategy (8 NeuronCores, data-parallel over batch, 64 rows/core):
  * Embedding lookup and x-projection fold into one host-precomputed table
        M2 = embed_table @ W_cell[:512] + b_cell  (+1 on the f columns)
    so the per-step x contribution is a row-gather of M2 by token id,
    injected into PSUM with a k=128 selection matmul that also seeds the
    accumulation groups (start=True).
  * Per-core batch is 64 = half the PE output partitions; hidden units are
    split in half across PSUM partition ranges: partitions 0:64 = (batch,
    units 0:512), 64:128 = (batch, units 512:1024).  The two column-groups
    of the PE run concurrently, giving full 128x128 utilization.
  * The epilogue tail (tanh_c, sig_o, h, transpose, bf16 copy) is chunked
    into 4x128-column slices so the first chunk of next step's stationary
    h^T is ready ~1us after the last gate matmul; the tile scheduler
    (readiness-driven) then keeps the PE nearly gapless, which also keeps
    the HAM clock-gate at 2.4 GHz.
  * Image encoder is emitted after the loop but depends only on inputs, so
    the scheduler uses its matmuls as PE gap-filler during the recurrence.
  * images are transposed on the host; all large weights live in SBUF,
    DMA'd up-front across three HWDGE queues (sync/scalar/vector).
"""

import os
import numpy as np

B, T = 512, 128
HIDDEN = 1024
VOCAB = 1024
EMB = 512
OUT = 1024
D_IMG = 2048
NCORES = 8
BS = B // NCORES  # 64 batch rows per core
HH = HIDDEN // 2  # 512 = per-half hidden units

_CACHE = {}


def _build_nc(n_steps: int):
    import concourse.bass as bass
    import concourse.mybir as mybir
    from concourse import bacc, tile

    f32 = mybir.dt.float32
    f32r = mybir.dt.float32r
    bf16 = mybir.dt.bfloat16
    i32 = mybir.dt.int32
    AF = mybir.ActivationFunctionType

    nc = bacc.Bacc("TRN2", target_bir_lowering=False, debug=False)

    m2p_d = nc.declare_dram_parameter("m2p", [2 * VOCAB, HH * 4], bf16, isOutput=False)
    wh_d = nc.declare_dram_parameter("wh", [HIDDEN, 4 * HIDDEN], bf16, isOutput=False)
    msg2_d = nc.declare_dram_parameter("msg2", [2 * BS, T], i32, isOutput=False)
    sfull_d = nc.declare_dram_parameter("sfull", [2 * BS, 2 * BS], bf16, isOutput=False)
    ident_d = nc.declare_dram_parameter("ident", [128, 128], bf16, isOutput=False)
    imt_d = nc.declare_dram_parameter("imt", [128, D_IMG // 2], bf16, isOutput=False)
    wimg_d = nc.declare_dram_parameter("wimg", [D_IMG, OUT], bf16, isOutput=False)
    whid_d = nc.declare_dram_parameter("whid", [HIDDEN, OUT], bf16, isOutput=False)
    o2_d = nc.declare_dram_parameter("o2", [2, 128], f32r, isOutput=False)
    bimg2_d = nc.declare_dram_parameter("bimg2", [2, OUT // 2], f32r, isOutput=False)
    bhid2_d = nc.declare_dram_parameter("bhid2", [2, OUT // 2], f32r, isOutput=False)
    oimg_d = nc.declare_dram_parameter("oimg", [128, OUT // 2], f32, isOutput=True)
    ohid_d = nc.declare_dram_parameter("ohid", [128, OUT // 2], f32, isOutput=True)

    with tile.TileContext(nc) as tc:
        with (
            tc.tile_pool(name="wpool", bufs=1) as wpool,
            tc.tile_pool(name="const", bufs=1) as cpool,
            tc.tile_pool(name="xg", bufs=3) as xgpool,
            tc.tile_pool(name="state", bufs=2) as stpool,
            tc.tile_pool(name="act", bufs=1) as apool,
            tc.tile_pool(name="outs", bufs=1) as opool,
            tc.tile_pool(name="psum", bufs=1, space="PSUM") as pspool,
        ):
            # ---- constants / small inputs (sync queue, first) ----
            msg2 = cpool.tile([2 * BS, T], i32, tag="msg2")
            nc.sync.dma_start(msg2[:], msg2_d[:])
            sfull = cpool.tile([2 * BS, 2 * BS], bf16, tag="sfull")
            nc.sync.dma_start(sfull[:], sfull_d[:])
            ident = cpool.tile([128, 128], bf16, tag="ident")
            nc.sync.dma_start(ident[:], ident_d[:])
            o2 = cpool.tile([2, 128], f32r, tag="o2")
            nc.sync.dma_start(o2[:], o2_d[:])
            bimg2 = cpool.tile([2, OUT // 2], f32r, tag="bimg2")
            nc.sync.dma_start(bimg2[:], bimg2_d[:])
            bhid2 = cpool.tile([2, OUT // 2], f32r, tag="bhid2")
            nc.sync.dma_start(bhid2[:], bhid2_d[:])
            imT = cpool.tile([128, D_IMG // 2], bf16, tag="imT")
            nc.sync.dma_start(imT[:], imt_d[:])

            # ---- W_h resident in SBUF: 8 chunks of [128, 4096], spread
            # across the three HWDGE queues so chunks land early ----
            wh_sb = []
            qeng = [nc.scalar, nc.sync]
            for ci in range(8):
                wt = wpool.tile([128, 4 * HIDDEN], bf16, tag=f"wh{ci}")
                qeng[ci % 2].dma_start(wt[:], wh_d[128 * ci : 128 * (ci + 1), :])
                wh_sb.append(wt)

            # ---- encoder weights resident (sync queue, after W_h) ----
            whid_sb = []
            for ci in range(8):
                wt = wpool.tile([128, OUT], bf16, tag=f"whid{ci}")
                nc.sync.dma_start(wt[:], whid_d[128 * ci : 128 * (ci + 1), :])
                whid_sb.append(wt)
            wimg_sb = []
            for ci in range(16):
                wt = wpool.tile([128, OUT], bf16, tag=f"wimg{ci}")
                nc.sync.dma_start(wt[:], wimg_d[128 * ci : 128 * (ci + 1), :])
                wimg_sb.append(wt)

            # ---- LSTM state init ----
            c_prev = stpool.tile([128, HH], f32, tag="c")
            nc.vector.memset(c_prev[:], 0.0)

            # bank order (i, g, f, o): g completes at the 50% mark so the
            # c chain (tanh_g -> m1 -> add -> tanh_c) finishes before the o
            # bank does; only sig_o -> h -> transpose -> cast remain on the
            # tail after the last matmul.
            GATE_OF_BANK = [0, 1, 2, 3]  # bank b holds W_cell gate column
            gate_sl = [slice(HH * b, HH * (b + 1)) for b in range(4)]

            def hT_sl(hT, ci):
                # transposed-block layout: block q = transpose of h cols
                # 128q:128(q+1); its cols 0:64 = half-A units (K-chunk q),
                # cols 64:128 = half-B units (K-chunk q+4)
                q, hi = (ci - 4, 64) if ci >= 4 else (ci, 0)
                return hT[:, 128 * q + hi : 128 * q + hi + 64]

            def imT_sl(ci):
                q, hi = (ci - 8, 64) if ci >= 8 else (ci, 0)
                return imT[:, 128 * q + hi : 128 * q + hi + 64]

            # per-bank PSUM tiles (1 bank each): a gate's activation waits
            # only for ITS bank's matmuls; subtile deps make the chunked
            # tail ops pipeline at 128-column granularity.
            gpb = [
                pspool.tile([128, HH], f32, tag=f"gp{b}", name=f"gp{b}")
                for b in range(4)
            ]
            # the o bank is double-buffered by step parity so its X-pass
            # matmul doesn't wait for the previous step's sig_o reads
            gp3b = pspool.tile([128, HH], f32, tag="gp3b", name="gp3b")
            gp3_par = [gpb[3], gp3b]
            tp = pspool.tile([128, 8 * BS], bf16, tag="tp")

            xg_tiles = {}

            def gather(t):
                xg = xgpool.tile([2 * BS, 4 * HH], bf16, tag="xg", name=f"xg{t}")
                nc.gpsimd.indirect_dma_start(
                    out=xg[:],
                    out_offset=None,
                    in_=m2p_d[:],
                    in_offset=bass.IndirectOffsetOnAxis(ap=msg2[:, t : t + 1], axis=0),
                )
                xg_tiles[t] = xg

            def bank_tile(t, b):
                return gp3_par[t % 2] if b == 3 else gpb[b]

            def xpass(t, banks):
                xg = xg_tiles[t]
                for b in banks:
                    nc.tensor.matmul(
                        out=bank_tile(t, b)[:],
                        lhsT=sfull[:],
                        rhs=xg[:, gate_sl[b]],
                        start=True,
                        stop=(t == 0),
                        skip_group_check=True,
                    )

            def recurrence(t, hT_cur):
                # h @ W_h: bank-outer so banks complete (and free for the
                # epilogue) in stagger; A/B column groups run concurrently.
                for b in range(4):
                    g = GATE_OF_BANK[b]
                    out = bank_tile(t, b)
                    for ci in range(8):
                        last = ci == 7
                        lhs = hT_sl(hT_cur, ci)
                        nc.tensor.matmul(
                            out=out[0:64, :],
                            lhsT=lhs,
                            rhs=wh_sb[ci][:, 1024 * g : 1024 * g + 512],
                            start=False,
                            stop=last,
                            skip_group_check=True,
                        )
                        nc.tensor.matmul(
                            out=out[64:128, :],
                            lhsT=lhs,
                            rhs=wh_sb[ci][:, 1024 * g + 512 : 1024 * (g + 1)],
                            start=False,
                            stop=last,
                            skip_group_check=True,
                        )

            # ---- prologue: first gathers + step-0 X pass ----
            gather(0)
            gather(1)
            xpass(0, range(4))

            hT_cur = None
            # ---- recurrence loop ----
            for t in range(n_steps):
                if t + 2 < n_steps:
                    gather(t + 2)

                # epilogue of step t (banks: 0=i, 1=g, 2=f, 3=o)
                sigi = apool.tile([128, HH], f32, tag="sigi")
                nc.scalar.activation(sigi[:], gpb[0][:], AF.Sigmoid)
                tanhg = apool.tile([128, HH], f32, tag="tanhg")
                m1 = apool.tile([128, HH], f32, tag="m1")
                for hf in range(2):
                    sl = slice(256 * hf, 256 * (hf + 1))
                    nc.scalar.activation(tanhg[:, sl], gpb[1][:, sl], AF.Tanh)
                    nc.vector.tensor_mul(m1[:, sl], sigi[:, sl], tanhg[:, sl])
                c_new = stpool.tile([128, HH], f32, tag="c", name=f"c{t}")
                cmul = apool.tile([128, HH], f32, tag="cmul")
                sigf = apool.tile([128, HH], f32, tag="sigf")
                tanhc = apool.tile([128, HH], f32, tag="tanhc")
                for hf in range(2):
                    sl = slice(256 * hf, 256 * (hf + 1))
                    nc.scalar.activation(sigf[:, sl], gpb[2][:, sl], AF.Sigmoid)
                    nc.vector.tensor_mul(cmul[:, sl], sigf[:, sl], c_prev[:, sl])
                    nc.vector.tensor_add(c_new[:, sl], cmul[:, sl], m1[:, sl])
                    nc.scalar.activation(tanhc[:, sl], c_new[:, sl], AF.Tanh)
                sigo = apool.tile([128, HH], f32, tag="sigo")
                h = apool.tile([128, HH], bf16, tag="h")
                for q in range(4):
                    sl = slice(128 * q, 128 * (q + 1))
                    nc.scalar.activation(sigo[:, sl], bank_tile(t, 3)[:, sl], AF.Sigmoid)
                    nc.vector.tensor_mul(h[:, sl], sigo[:, sl], tanhc[:, sl])

                # PE: seed all four of next step's banks (these fill the PE
                # while sig_o/h chunks compute), then transpose h (chunked),
                # then next step's recurrence.
                if t + 1 < n_steps:
                    xpass(t + 1, range(4))
                for q in range(4):
                    nc.tensor.transpose(
                        out=tp[:, 128 * q : 128 * (q + 1)],
                        in_=h[:, 128 * q : 128 * (q + 1)],
                        identity=ident[:],
                    )
                hT_next = stpool.tile([128, 8 * BS], bf16, tag="hT", name=f"hT{t}")
                for q in range(4):
                    sl = slice(128 * q, 128 * (q + 1))
                    nc.vector.tensor_copy(hT_next[:, sl], tp[:, sl])
                if t + 1 < n_steps:
                    recurrence(t + 1, hT_next)

                c_prev = c_new
                hT_cur = hT_next

            # ---- hidden encoder: out = h @ W_hid + b_hid ----
            ohp = pspool.tile([128, OUT // 2], f32, tag="ohp")
            nc.tensor.matmul(
                out=ohp[:], lhsT=o2[:], rhs=bhid2[:],
                start=True, stop=False, skip_group_check=True,
            )
            for ci in range(8):
                last = ci == 7
                lhs = hT_sl(hT_cur, ci)
                nc.tensor.matmul(
                    out=ohp[0:64, :], lhsT=lhs, rhs=whid_sb[ci][:, 0:512],
                    start=False, stop=last, skip_group_check=True,
                )
                nc.tensor.matmul(
                    out=ohp[64:128, :], lhsT=lhs, rhs=whid_sb[ci][:, 512:1024],
                    start=False, stop=last, skip_group_check=True,
                )
            ohid_sb = opool.tile([128, OUT // 2], f32, tag="ohid")
            nc.vector.tensor_copy(ohid_sb[:], ohp[:])
            nc.sync.dma_start(ohid_d[:], ohid_sb[:])

            # ---- images encoder: depends only on inputs; the scheduler
            # runs these matmuls whenever the PE would otherwise idle ----
            oip = pspool.tile([128, OUT // 2], f32, tag="oip")
            nc.tensor.matmul(
                out=oip[:], lhsT=o2[:], rhs=bimg2[:],
                start=True, stop=False, skip_group_check=True,
            )
            for ci in range(16):
                last = ci == 15
                lhs = imT_sl(ci)
                nc.tensor.matmul(
                    out=oip[0:64, :], lhsT=lhs, rhs=wimg_sb[ci][:, 0:512],
                    start=False, stop=last, skip_group_check=True,
                )
                nc.tensor.matmul(
                    out=oip[64:128, :], lhsT=lhs, rhs=wimg_sb[ci][:, 512:1024],
                    start=False, stop=last, skip_group_check=True,
                )
            oimg_sb = opool.tile([128, OUT // 2], f32, tag="oimg")
            nc.vector.tensor_copy(oimg_sb[:], oip[:])
            nc.sync.dma_start(oimg_d[:], oimg_sb[:])

    nc.compile()
    return nc


def _host_prep(images, embed_table, W_cell, b_cell, W_img, b_img, W_hid, b_hid,
               message):
    """Builds the per-core input maps (all host-side preprocessing)."""
    from ml_dtypes import bfloat16

    W_x = W_cell[:EMB]          # [512, 4096]
    W_h = np.ascontiguousarray(W_cell[EMB:]).astype(bfloat16)  # [1024, 4096]

    M2 = embed_table.astype(np.float32) @ W_x + b_cell  # [1024, 4096]
    M2[:, 2 * HIDDEN : 3 * HIDDEN] += 1.0  # fold the f-gate +1.0
    # row 2v+h = [i_h, g_h, f_h, o_h] halves of vocab row v (bank order)
    M2p = np.ascontiguousarray(
        M2.reshape(VOCAB, 4, 2, HH).transpose(0, 2, 1, 3).reshape(2 * VOCAB, 4 * HH)
    ).astype(bfloat16)

    sfull = np.zeros((2 * BS, 2 * BS), np.float32)
    for m in range(BS):
        sfull[2 * m, m] = 1.0
        sfull[2 * m + 1, BS + m] = 1.0
    sfull = sfull.astype(bfloat16)

    ident = np.eye(128, dtype=bfloat16)

    o2 = np.zeros((2, 128), np.float32)
    o2[0, 0:64] = 1.0
    o2[1, 64:128] = 1.0

    W_img_b = W_img.astype(bfloat16)
    W_hid_b = W_hid.astype(bfloat16)
    bimg2 = np.stack([b_img[: OUT // 2], b_img[OUT // 2 :]]).astype(np.float32)
    bhid2 = np.stack([b_hid[: OUT // 2], b_hid[OUT // 2 :]]).astype(np.float32)

    in_maps = []
    for core in range(NCORES):
        sl = slice(core * BS, (core + 1) * BS)
        msg = message[sl]  # [64, T] int32
        msg2 = np.empty((2 * BS, T), np.int32)
        msg2[0::2] = 2 * msg
        msg2[1::2] = 2 * msg + 1
        # host-side image transpose into the packed-block layout:
        # imT[k, 128q + hi + m] = images[m, 1024*hi/64 + 128q + k]
        A = images[sl].reshape(BS, 2, 8, 128)  # [m, half, q, k]
        imT = np.ascontiguousarray(
            A.transpose(3, 2, 1, 0).reshape(128, D_IMG // 2)
        ).astype(bfloat16)
        in_maps.append(
            {
                "m2p": M2p,
                "wh": W_h,
                "msg2": msg2,
                "sfull": sfull,
                "ident": ident,
                "imt": imT,
                "wimg": W_img_b,
                "whid": W_hid_b,
                "o2": o2,
                "bimg2": bimg2,
                "bhid2": bhid2,
            }
        )
    return in_maps


def kernel(images, embed_table, W_cell, b_cell, W_img, b_img, W_hid, b_hid,
           message):
    import sys
    if "/opt/trn_rl_repo" not in sys.path:
        sys.path.insert(0, "/opt/trn_rl_repo")
    from concourse.bass_utils import run_bass_kernel_spmd

    images = np.asarray(images, np.float32)
    embed_table = np.asarray(embed_table, np.float32)
    W_cell = np.asarray(W_cell, np.float32)
    b_cell = np.asarray(b_cell, np.float32)
    W_img = np.asarray(W_img, np.float32)
    b_img = np.asarray(b_img, np.float32)
    W_hid = np.asarray(W_hid, np.float32)
    b_hid = np.asarray(b_hid, np.float32)
    message = np.asarray(message, np.int32)

    n_steps = T
    if "nc" not in _CACHE or _CACHE.get("n_steps") != n_steps:
        _CACHE["nc"] = _build_nc(n_steps)
        _CACHE["n_steps"] = n_steps
    nc = _CACHE["nc"]

    in_maps = _host_prep(
        images, embed_table, W_cell, b_cell, W_img, b_img, W_hid, b_hid, message
    )
    res = run_bass_kernel_spmd(nc, in_maps, core_ids=list(range(NCORES)))
    results = res.results

    images_encoded = np.empty((B, OUT), np.float32)
    hidden_encoded = np.empty((B, OUT), np.float32)
    for core in range(NCORES):
        sl = slice(core * BS, (core + 1) * BS)
        oi = results[core]["oimg"]
        oh = results[core]["ohid"]
        images_encoded[sl, : OUT // 2] = oi[0:64]
        images_encoded[sl, OUT // 2 :] = oi[64:128]
        hidden_encoded[sl, : OUT // 2] = oh[0:64]
        hidden_encoded[sl, OUT // 2 :] = oh[64:128]
    return images_encoded, hidden_encoded
